# revision 1
# baseline (speedup 1.0000x reference)
"""ATTConv (GNN bilinear-attention message passing) on 8 Trainium2 NeuronCores.

Strategy (self-contained: full inputs in, full output out):
  * Host sorts edges by dst and shards them by dst-node range across the 8
    cores (12500 nodes/core) -> all segment reductions are core-local, no
    collectives.
  * Algebraic refold: a_e = theta(x_s).phi(x_d) = x_s . q_d with
    q = x @ (W_theta^T W_phi)^T; only the 128x128 weight product is formed
    on host, q itself is computed on-device.  The weighted aggregation is
    z_n = (1/den_n) * (sum_e exp(a_e) x_src_e) @ W_fc^T, so W_fc applies
    after aggregation and only x rows are gathered per edge.
  * Nodes are grouped in 128-node windows (98/core); windows in supertiles
    of 4.  Edges of each window are grouped by src-quarter (dma_gather has
    int16 indices, so x is passed as 4 sub-tables of 25000 rows) and padded
    to 128-edge chunks; chunk counts are equalized across cores so the SPMD
    program is identical.
  * Per supertile: 4 dma_gather ops fetch x[src] (fp16) and 1 fetches
    q[dst] (fp16) for ~10k edges at once (SWDGE fixed cost amortized).
    Per 128-edge chunk: DVE dot-product + ACT exp, a fused tensor_scalar
    builds the scaled one-hot S'_T[e,slot] = (iota==dst_rel)*exp(a), and two
    PSUM-accumulating matmuls produce the window aggregate G^T[f,slot] and
    denominators.  Window epilogue: G^T @ W_fc^T, divide by den, LayerNorm,
    gamma/beta, DMA the 128 output rows.
"""
import os
import sys

for _p in ("/opt/trn_rl_repo", "/root/.axon_site/_ro/trn_rl_repo"):
    if os.path.isdir(_p):
        if _p not in sys.path:
            sys.path.insert(0, _p)
        break

import numpy as np

import concourse.bass as bass
import concourse.bacc as bacc
import concourse.tile as tile
from concourse import mybir
from concourse.bass_utils import run_bass_kernel_spmd

P = 128
N = 100000
E = 1600000
D = 128
NCORE = 8
NLOC = N // NCORE            # 12500 nodes per core
NWIN = (NLOC + P - 1) // P   # 98 windows per core
NPAD = NWIN * P              # 12544
NQUART = 4
QROWS = N // NQUART          # 25000 rows per x sub-table (int16-safe)
SUPER = 4                    # windows per supertile
SCALE = float(D) ** -0.5
LN_EPS = 1e-5
F32 = mybir.dt.float32
F16 = mybir.dt.float16
I16 = mybir.dt.int16
PAD_REL = 1000.0             # one-hot miss value for padded slots


def _supertiles(nwin):
    return [list(range(s, min(s + SUPER, nwin))) for s in range(0, nwin, SUPER)]


def build_kernel(cwr, nloc=NLOC, nwin=NWIN, qrows=QROWS, passes=1):
    """cwr: [nwin][NQUART] chunk counts per (window, quarter), uniform
    across cores."""
    npad = nwin * P
    nchunks = int(sum(sum(r) for r in cwr))
    nedge = nchunks * P

    nc = bacc.Bacc("TRN2", target_bir_lowering=False, debug=False,
                   enable_asserts=False, num_devices=NCORE,
                   num_swdge_queues=4)

    xq = [nc.dram_tensor(f"x_q{r}", [qrows, P], F16, kind="ExternalInput")
          for r in range(NQUART)]
    xlT = nc.dram_tensor("xlocT", [P, npad], F16, kind="ExternalInput")
    xidx = nc.dram_tensor("xidx_w", [P, nedge // 16], I16, kind="ExternalInput")
    qidx = nc.dram_tensor("qidx_w", [P, nedge // 16], I16, kind="ExternalInput")
    dreld = nc.dram_tensor("drel_p", [P, nchunks], F32, kind="ExternalInput")
    mqT = nc.dram_tensor("mqT", [P, P], F16, kind="ExternalInput")
    wfcT = nc.dram_tensor("wfcT", [P, P], F32, kind="ExternalInput")
    gbc = nc.dram_tensor("gamma_bc", [P, P], F32, kind="ExternalInput")
    bbc = nc.dram_tensor("beta_bc", [P, P], F32, kind="ExternalInput")
    zout = nc.dram_tensor("zout", [nloc, P], F32, kind="ExternalOutput")
    qt = nc.dram_tensor("q_tab", [npad, P], F16, kind="Internal")

    iota_f = np.tile(np.arange(P, dtype=np.float16)[None, :], (P, 1))
    iota_f_d = nc.inline_tensor(iota_f, "iota_f")

    # static chunk schedule ---------------------------------------------
    # global chunk order: supertile -> quarter -> window -> chunk
    sts = _supertiles(nwin)
    chunk_of = []          # (w, r, st_idx)
    x_gathers = []         # (st_idx, r, n_idx, first_chunk_global)
    q_gathers = []         # (st_idx, n_idx, first_chunk_global)
    for si, ws in enumerate(sts):
        st_first = len(chunk_of)
        for r in range(NQUART):
            n_idx = sum(cwr[w][r] for w in ws) * P
            if n_idx:
                x_gathers.append((si, r, n_idx, len(chunk_of)))
            for w in ws:
                for _ in range(cwr[w][r]):
                    chunk_of.append((w, r, si))
        n_st = (len(chunk_of) - st_first) * P
        q_gathers.append((si, n_st, st_first))
    assert len(chunk_of) == nchunks
    # stream position of each (window, quarter) group's first chunk
    group_first = {}
    for ci, (w, r, si) in enumerate(chunk_of):
        group_first.setdefault((w, r), ci)

    with tile.TileContext(nc) as tc:
        with (
            tc.tile_pool(name="const", bufs=1) as cpool,
            tc.tile_pool(name="psz", bufs=2, space="PSUM") as psz,
        ):
            IF = cpool.tile([P, P], F16, tag="if")
            nc.sync.dma_start(IF[:], iota_f_d.ap()[:, :])
            MQ = cpool.tile([P, P], F16, tag="mq")
            nc.sync.dma_start(MQ[:], mqT.ap()[:, :])
            WT = cpool.tile([P, P], F32, tag="wt")
            nc.sync.dma_start(WT[:], wfcT.ap()[:, :])
            GB = cpool.tile([P, P], F32, tag="gb")
            nc.sync.dma_start(GB[:], gbc.ap()[:, :])
            BB = cpool.tile([P, P], F32, tag="bb")
            nc.sync.dma_start(BB[:], bbc.ap()[:, :])
            ones16 = cpool.tile([P, 1], F16, tag="ones16")
            nc.vector.memset(ones16[:], 1.0)
            eps_col = cpool.tile([P, 1], F32, tag="eps_c")
            nc.vector.memset(eps_col[:], LN_EPS)
            drel_all = cpool.tile([P, nchunks], F32, tag="drel")
            nc.sync.dma_start(drel_all[:], dreld.ap()[:, :])

            # ---- phase 0: q_tab = (x_loc @ Mq^T) in fp16 ----
            with (tc.tile_pool(name="ph0", bufs=1) as p0,
                  tc.tile_pool(name="ph0w", bufs=3) as p0w):
                xlT_sb = p0.tile([P, npad], F16, tag="xlt")
                nc.sync.dma_start(xlT_sb[:], xlT.ap()[:, :])
                for w in range(nwin):
                    q_ps = psz.tile([P, P], F32, tag="z")
                    nc.tensor.matmul(out=q_ps[:], lhsT=xlT_sb[:, w*P:(w+1)*P],
                                     rhs=MQ[:], start=True, stop=True)
                    q_sb = p0w.tile([P, P], F16, tag="qsb")
                    nc.vector.tensor_copy(q_sb[:], q_ps[:])
                    nc.sync.dma_start(qt.ap()[w*P:(w+1)*P, :], q_sb[:])

            # ---- main loop ----
            with (
                tc.tile_pool(name="idx", bufs=2) as ipool,
                tc.tile_pool(name="gx", bufs=8) as gxpool,
                tc.tile_pool(name="gq", bufs=2) as gqpool,
                tc.tile_pool(name="wk", bufs=4) as wpool,
                tc.tile_pool(name="ep", bufs=2) as epool,
                tc.tile_pool(name="psg", bufs=4, space="PSUM") as psg,
                tc.tile_pool(name="psd", bufs=2, space="PSUM") as psd,
            ):
                for pass_ in range(passes):
                  for si, ws in enumerate(sts):
                    # index tiles for this supertile
                    st_x = [g for g in x_gathers if g[0] == si]
                    st_q = q_gathers[si]
                    n_st = st_q[1]
                    st_first = st_q[2]

                    xcols = n_st // 16
                    xi_sb = ipool.tile([P, xcols], I16, tag="xi")
                    nc.sync.dma_start(
                        xi_sb[:], xidx.ap()[:, st_first*8:st_first*8 + xcols])
                    qi_sb = ipool.tile([P, xcols], I16, tag="qi")
                    nc.sync.dma_start(
                        qi_sb[:], qidx.ap()[:, st_first*8:st_first*8 + xcols])

                    # gathers: Q split in 4 queue-balanced pieces (queues
                    # 0-3), X quarters on queues 1-3; SWDGE desc-gen
                    # pipelines with SDMA drain per queue.
                    Q_st = gqpool.tile([P, (n_st // P) * P], F16, tag="qg")
                    bq = n_st // P
                    qsplit = [(k * bq // 4, (k + 1) * bq // 4) for k in range(4)]
                    for k, (b0, b1) in enumerate(qsplit):
                        if b1 == b0:
                            continue
                        nq_i = (b1 - b0) * P
                        nc.gpsimd.dma_gather(
                            out_ap=Q_st[:, b0*P:b1*P].rearrange(
                                "p (b e) -> p b e", e=P),
                            in_ap=qt.ap()[:, :],
                            idxs_ap=qi_sb[:, b0*8:b0*8 + nq_i // 16],
                            num_idxs=nq_i, num_idxs_reg=nq_i, elem_size=P,
                            single_packet=False, queue_num=k)
                    X_st = {}
                    for (_, r, n_idx, fc) in st_x:
                        xt_sb = gxpool.tile([P, (n_idx // P) * P], F16, tag="xg")
                        off = (fc - st_first) * 8
                        nc.gpsimd.dma_gather(
                            out_ap=xt_sb[:].rearrange("p (b e) -> p b e", e=P),
                            in_ap=xq[r].ap()[:, :],
                            idxs_ap=xi_sb[:, off:off + n_idx // 16],
                            num_idxs=n_idx, num_idxs_reg=n_idx, elem_size=P,
                            single_packet=False, queue_num=1 + (r % 3))
                        X_st[r] = (xt_sb, fc)

                    # exp-scores per quarter slab: one mul + one strided
                    # reduce + one exp over ~20 chunks at a time
                    E_st = {}
                    for (_, r, n_idx, fc) in st_x:
                        xt_sb, _ = X_st[r]
                        cb = (fc - st_first) * P
                        scr = wpool.tile([P, n_idx], F16, tag="scr",
                                         name=f"scr_{si}_{r}")
                        nc.vector.tensor_tensor(
                            out=scr[:], in0=xt_sb[:],
                            in1=Q_st[:, cb:cb + n_idx],
                            op=mybir.AluOpType.mult)
                        a_sb = wpool.tile([P, n_idx // P], F32, tag="a",
                                          name=f"a_{si}_{r}")
                        nc.vector.tensor_reduce(
                            out=a_sb[:],
                            in_=scr[:].rearrange("p (b e) -> p b e", e=P),
                            axis=mybir.AxisListType.X,
                            op=mybir.AluOpType.add)
                        e_sb = wpool.tile([P, n_idx // P], F32, tag="e",
                                          name=f"e_{si}_{r}")
                        nc.scalar.activation(e_sb[:], a_sb[:],
                                             mybir.ActivationFunctionType.Exp,
                                             scale=SCALE)
                        E_st[r] = (e_sb, fc)

                    # compute window-major; map each chunk to its stream slot
                    Gt_w = {}
                    Den_all = psd.tile([P, len(ws)], F32, tag="d")
                    for wl, w in enumerate(ws):
                        Gt = psg.tile([P, P], F32, tag="g", name=f"gt_{w}")
                        Gt_w[w] = Gt
                        wchunks = [(r, k) for r in range(NQUART)
                                   for k in range(cwr[w][r])]
                        for ki, (r, k) in enumerate(wchunks):
                            ci = group_first[(w, r)] + k
                            xt_sb, fc = X_st[r]
                            e_sb, _ = E_st[r]
                            xb = ci - fc
                            Xs = xt_sb[:, xb*P:(xb+1)*P]

                            SpT = wpool.tile([P, P], F16, tag="spt")
                            nc.vector.tensor_scalar(
                                out=SpT[:], in0=IF[:],
                                scalar1=drel_all[:, ci:ci+1],
                                scalar2=e_sb[:, xb:xb+1],
                                op0=mybir.AluOpType.is_equal,
                                op1=mybir.AluOpType.mult)

                            first = ki == 0
                            last = ki == len(wchunks) - 1
                            nc.tensor.matmul(out=Gt[:], lhsT=Xs, rhs=SpT[:],
                                             start=first, stop=last)
                            nc.tensor.matmul(out=Den_all[:, wl:wl+1],
                                             lhsT=SpT[:], rhs=ones16[:],
                                             start=first, stop=last)

                    # ---- window epilogues, batched per supertile ----
                    for wl, w in enumerate(ws):
                        Den = Den_all[:, wl:wl+1]
                        Gt_sb = epool.tile([P, P], F32, tag="gt")
                        nc.vector.tensor_copy(Gt_sb[:], Gt_w[w][:])
                        Z = psz.tile([P, P], F32, tag="z")
                        nc.tensor.matmul(out=Z[:], lhsT=Gt_sb[:], rhs=WT[:],
                                         start=True, stop=True)
                        den_c = epool.tile([P, 1], F32, tag="denc")
                        nc.vector.tensor_scalar(
                            out=den_c[:], in0=Den[:],
                            scalar1=1e-30, scalar2=None,
                            op0=mybir.AluOpType.max)
                        recip = epool.tile([P, 1], F32, tag="recip")
                        nc.vector.reciprocal(recip[:], den_c[:])
                        z_sb = epool.tile([P, P], F32, tag="zsb")
                        nc.vector.tensor_scalar(
                            out=z_sb[:], in0=Z[:], scalar1=recip[:, :1],
                            scalar2=None, op0=mybir.AluOpType.mult)
                        mu = epool.tile([P, 1], F32, tag="mu")
                        nc.vector.tensor_reduce(out=mu[:], in_=z_sb[:],
                                                axis=mybir.AxisListType.X,
                                                op=mybir.AluOpType.add)
                        mu_m = epool.tile([P, 1], F32, tag="mum")
                        nc.scalar.mul(mu_m[:], mu[:], 1.0 / P)
                        xc = epool.tile([P, P], F32, tag="xc")
                        nc.vector.tensor_scalar(
                            out=xc[:], in0=z_sb[:], scalar1=mu_m[:, :1],
                            scalar2=None, op0=mybir.AluOpType.subtract)
                        sq = epool.tile([P, P], F32, tag="sq")
                        nc.vector.tensor_tensor(out=sq[:], in0=xc[:], in1=xc[:],
                                                op=mybir.AluOpType.mult)
                        var = epool.tile([P, 1], F32, tag="var")
                        nc.vector.tensor_reduce(out=var[:], in_=sq[:],
                                                axis=mybir.AxisListType.X,
                                                op=mybir.AluOpType.add)
                        std = epool.tile([P, 1], F32, tag="std")
                        nc.scalar.activation(std[:], var[:],
                                             mybir.ActivationFunctionType.Sqrt,
                                             scale=1.0 / P, bias=eps_col[:, :1])
                        rstd = epool.tile([P, 1], F32, tag="rstd")
                        nc.vector.reciprocal(rstd[:], std[:])
                        nrm = epool.tile([P, P], F32, tag="nrm")
                        nc.vector.tensor_scalar(
                            out=nrm[:], in0=xc[:], scalar1=rstd[:, :1],
                            scalar2=None, op0=mybir.AluOpType.mult)
                        og = epool.tile([P, P], F32, tag="og")
                        nc.vector.tensor_tensor(out=og[:], in0=nrm[:], in1=GB[:],
                                                op=mybir.AluOpType.mult)
                        ob = epool.tile([P, P], F32, tag="ob")
                        nc.vector.tensor_tensor(out=ob[:], in0=og[:], in1=BB[:],
                                                op=mybir.AluOpType.add)
                        nw_rows = min(P, nloc - w * P)
                        nc.sync.dma_start(zout.ap()[w*P:w*P+nw_rows, :],
                                          ob[:nw_rows, :])
    nc.compile()
    return nc


def _wrap16(idx_flat):
    """edge j -> partition j%16, column j//16; replicated to 128 parts."""
    s = len(idx_flat) // 16
    base = idx_flat.reshape(s, 16).T.astype(np.int16)   # [16, s]
    return np.tile(base, (8, 1))


def preprocess(x, W_fc, W_theta, W_phi, gamma, beta, src, dst,
               n=N, ncore=NCORE, nloc=NLOC, nwin=NWIN, qrows=QROWS):
    src = np.ascontiguousarray(src, dtype=np.int64)
    dst = np.ascontiguousarray(dst, dtype=np.int64)
    order = np.argsort(dst, kind="stable")
    src_s = src[order]
    dst_s = dst[order]
    bounds = np.searchsorted(dst_s, np.arange(0, n + 1, nloc))

    # per-core edge groups by (window, quarter)
    per_core = []
    ngroups = nwin * NQUART
    counts = np.zeros((ncore, ngroups), np.int64)
    for d in range(ncore):
        lo, hi = bounds[d], bounds[d + 1]
        dl = dst_s[lo:hi] - d * nloc
        sl = src_s[lo:hi]
        w = dl >> 7
        r = sl // qrows
        g = w * NQUART + r
        ord2 = np.argsort(g, kind="stable")
        per_core.append((dl[ord2], sl[ord2], g[ord2]))
        counts[d] = np.bincount(g, minlength=ngroups)

    # uniform chunk counts per (window, quarter) = max over cores
    cwr_flat = np.ceil(counts.max(axis=0) / P).astype(np.int64)
    cwr = cwr_flat.reshape(nwin, NQUART)
    assert (cwr.sum(axis=1) > 0).all()

    # static chunk -> stream slot mapping (same order build_kernel uses)
    sts = _supertiles(nwin)
    group_slot_off = np.zeros(ngroups, np.int64)   # edge-slot offset per group
    nslots = 0
    for ws in sts:
        for r in range(NQUART):
            for w in ws:
                g = w * NQUART + r
                group_slot_off[g] = nslots
                nslots += cwr[w][r] * P
    nchunks = nslots // P

    MqT = (W_phi.astype(np.float64).T @ W_theta.astype(np.float64)).astype(np.float16)
    WfcT = np.ascontiguousarray(W_fc.T, dtype=np.float32)
    GBC = np.tile(np.asarray(gamma, np.float32)[None, :], (P, 1))
    BBC = np.tile(np.asarray(beta, np.float32)[None, :], (P, 1))
    x = np.ascontiguousarray(x, dtype=np.float32)
    x16 = x.astype(np.float16)
    xq = {f"x_q{r}": np.ascontiguousarray(x16[r*qrows:(r+1)*qrows])
          for r in range(NQUART)}

    in_maps = []
    for d in range(ncore):
        dl, sl, g = per_core[d]
        j = np.arange(len(dl), dtype=np.int64)
        gstart = np.zeros(ngroups, np.int64)
        cnt = np.bincount(g, minlength=ngroups)
        gstart[1:] = np.cumsum(cnt)[:-1]
        slot = group_slot_off[g] + (j - gstart[g])

        xidx_f = np.zeros(nslots, np.int64)
        xidx_f[slot] = sl % qrows
        qidx_f = np.zeros(nslots, np.int64)
        qidx_f[slot] = dl
        drel_f = np.full(nslots, PAD_REL, np.float32)
        drel_f[slot] = (dl & 127).astype(np.float32)

        npadl = nwin * P
        xlocT = np.zeros((P, npadl), np.float16)
        xlocT[:, :nloc] = x16[d * nloc:(d + 1) * nloc].T

        in_maps.append({
            **xq,
            "xlocT": xlocT,
            "xidx_w": _wrap16(xidx_f),
            "qidx_w": _wrap16(qidx_f),
            "drel_p": np.ascontiguousarray(
                drel_f.reshape(nchunks, P).T),
            "mqT": MqT,
            "wfcT": WfcT,
            "gamma_bc": GBC,
            "beta_bc": BBC,
        })
    return in_maps, cwr


_cache = {}


def kernel(x, W_fc, W_theta, W_phi, gamma, beta, src, dst):
    in_maps, cwr = preprocess(np.asarray(x), np.asarray(W_fc),
                              np.asarray(W_theta), np.asarray(W_phi),
                              np.asarray(gamma), np.asarray(beta),
                              np.asarray(src), np.asarray(dst))
    key = cwr.tobytes()
    if key not in _cache:
        _cache[key] = build_kernel([list(map(int, row)) for row in cwr])
    nc = _cache[key]
    res = run_bass_kernel_spmd(nc, in_maps, core_ids=list(range(NCORE)))
    out = np.concatenate([res.results[c]["zout"] for c in range(NCORE)], axis=0)
    return np.ascontiguousarray(out, dtype=np.float32)

